# revision 20
# baseline (speedup 1.0000x reference)
"""BitLinear (RMSNorm + per-token int8 act fake-quant + ternary weight
fake-quant + linear) Trainium2 Bass kernel, data-parallel over 8 NeuronCores.

Strategy
--------
Tokens (B*S = 32768) are sharded 8 ways (4096/core); W replicated. The
matmul runs on the PE in fp8 DoubleRowSwInterleave mode (0.5 cycles/row,
two fp8 k-planes per instruction = 4x the bf16 MAC rate) with an EXACT
hi/lo split of the quantized activations:

    q = 16*qh + lo,   qh = RNE(q/16) in [-8,8],  lo in [-8,8]

Both planes are fp8e4-exact, pair (16qh, lo) shares one weight copy via a
stride-0 rhs pair dim, so the matmul is integer-exact at half the bf16 PE
time (218us -> PE-bound ~7us/tile).

Per 128-token tile [128, 2048] (x shipped as bf16, host row-reversed per
128-block to absorb the SwInterleave column reversal):
  ACT:  sumsq via Square+accum -> sqv = sqrt(ss/2048 + 1e-6)
  DVE:  rms = 1/sqv; mx = absmax(x); d = rms*mx + 1e-5; rcd = 1/d;
        s = (127*rcd)*rms
  ACT:  scr1 = x*s + C  (fma, C = 1.5*2^23: scr1 = C + q, exact RNE)
  DVE:  scr2 = scr1*2^-4 + 11796480  (= C + qh, exact magic at /16 grid)
        hi   = (scr2 - C)*16 -> fp8 lane0   (exact: multiples of 16)
  Pool: lo   = (scr1 - C) - hi -> fp8 lane1 (exact small ints)
  DMA:  xbar-transpose the packed (hi,lo) uint16 pairs -> k-major planes
  PE:   16 k-blocks x 4 out-groups SwInterleave matmuls, fp32 psum
        (integer-exact: |partial| < 2^19)
  ACT/Pool: psum -> bf16 (integer outputs, bf16-exact scale-free copy)
  DMA:  out tile -> DRAM

The per-token output scale s3 = (1/a)/w_scale is applied on HOST (it does
not affect which integers the device computes, only the final scaling, so
host/device ulp differences cannot flip any quantization decision).
Measured end-to-end rel err vs the fp32 reference: ~5.9e-3 (dominated by
bf16-x quantization boundary flips; the matmul itself is exact).
"""
import numpy as np
from contextlib import ExitStack

import concourse.bacc as bacc
import concourse.tile as tile
from concourse import mybir
from concourse.bass_utils import run_bass_kernel_spmd

F32 = mybir.dt.float32
BF16 = mybir.dt.bfloat16
FP8 = mybir.dt.float8e4
U16 = mybir.dt.uint16
AL = mybir.AluOpType
AF = mybir.ActivationFunctionType
AX = mybir.AxisListType
PM = mybir.MatmulPerfMode

B, S, DIN, DOUT = 4, 8192, 2048, 2048
NCORES = 8
TOK = B * S                  # 32768
TPC = TOK // NCORES          # 4096 tokens per core
NT = TPC // 128              # 32 token tiles per core
KB = DIN // 128              # 16 contraction blocks
OGW = 512                    # psum free dim per matmul (one bank)
OG = DOUT // OGW             # 4 output groups

C_MAGIC = 12582912.0         # 1.5*2^23: +C then -C rounds f32 to int (RNE)


_CACHE = {}


def _build(use_xg=False):
    nc = bacc.Bacc("TRN2", target_bir_lowering=False, debug=False,
                   num_devices=NCORES)
    x_d = nc.declare_dram_parameter("x", [TPC, DIN], BF16, isOutput=False)
    if use_xg:
        xg_d = nc.declare_dram_parameter("xg", [TPC, DIN], BF16,
                                         isOutput=False)
    wq_d = nc.declare_dram_parameter("wq", [128, KB * DOUT], FP8,
                                     isOutput=False)
    o_d = nc.declare_dram_parameter("out", [TPC, DOUT], BF16, isOutput=True)

    with tile.TileContext(nc) as tc:
        with ExitStack() as ctx:
            cst = ctx.enter_context(tc.tile_pool(name="cst", bufs=1))
            wqp = ctx.enter_context(tc.tile_pool(name="wqp", bufs=1))
            xp = ctx.enter_context(tc.tile_pool(name="xp", bufs=6))
            xgp = (ctx.enter_context(tc.tile_pool(name="xgp", bufs=3))
                   if use_xg else None)
            s0p = ctx.enter_context(tc.tile_pool(name="s0p", bufs=1))
            s1p = ctx.enter_context(tc.tile_pool(name="s1p", bufs=4))
            s2p = ctx.enter_context(tc.tile_pool(name="s2p", bufs=4))
            qp = ctx.enter_context(tc.tile_pool(name="qp", bufs=4))
            qtpool = ctx.enter_context(tc.tile_pool(name="qtp", bufs=6))
            op = ctx.enter_context(tc.tile_pool(name="op", bufs=4))
            st = ctx.enter_context(tc.tile_pool(name="st", bufs=10))
            psp = ctx.enter_context(tc.tile_pool(name="psp", bufs=2,
                                                 space="PSUM"))

            # ---- prefetch + constants ----
            NPRE = 4
            xpre = [xp.tile([128, DIN], BF16, name="xt", tag="xtile")
                    for _ in range(NPRE)]
            nc.sync.dma_start(out=xpre[0], in_=x_d[0:128, :])
            xgpre = []
            if use_xg:
                xgpre = [xgp.tile([128, DIN], BF16, name="xgt", tag="xgtile")
                         for _ in range(NPRE)]
                nc.sync.dma_start(out=xgpre[0], in_=xg_d[0:128, :])
            epsb = cst.tile([128, 1], F32, name="epsb")
            nc.vector.memset(epsb, 1e-6)
            eps5 = cst.tile([128, 1], F32, name="eps5")
            nc.vector.memset(eps5, 1e-5)
            cmag = cst.tile([128, 1], F32, name="cmag")
            nc.vector.memset(cmag, C_MAGIC)
            # warm the activation table entries used below
            warmt = cst.tile([128, 1], F32, name="warmt")
            nc.scalar.activation(out=warmt, in_=cmag, func=AF.Square)
            nc.scalar.activation(out=warmt, in_=cmag, func=AF.Sqrt)
            nc.scalar.activation(out=warmt, in_=cmag, func=AF.Identity)

            for it in range(1, NPRE):
                nc.sync.dma_start(out=xpre[it],
                                  in_=x_d[it * 128:(it + 1) * 128, :])
                if use_xg:
                    nc.sync.dma_start(out=xgpre[it],
                                      in_=xg_d[it * 128:(it + 1) * 128, :])

            # ---- replicated ternary weights, k-major fp8 ----
            wq = wqp.tile([128, KB, DOUT], FP8, name="wq")

            def dma_wq(kt):
                nc.sync.dma_start(out=wq[:, kt, :],
                                  in_=wq_d[:, kt * DOUT:(kt + 1) * DOUT])
            for kt in range(2):
                dma_wq(kt)

            # ---- token tiles ----
            prev = None
            for it in range(NT):
                if it < NPRE:
                    xt = xpre[it]
                    xgt = xgpre[it] if use_xg else xt
                else:
                    xt = xp.tile([128, DIN], BF16, name="xt", tag="xtile")
                    nc.sync.dma_start(out=xt,
                                      in_=x_d[it * 128:(it + 1) * 128, :])
                    if use_xg:
                        xgt = xgp.tile([128, DIN], BF16, name="xgt",
                                       tag="xgtile")
                        nc.sync.dma_start(
                            out=xgt, in_=xg_d[it * 128:(it + 1) * 128, :])
                    else:
                        xgt = xt

                # stats: ss = sum(x^2); mx = absmax(xg)
                scr0 = s0p.tile([128, DIN], BF16, name="scr0")
                ss = st.tile([128, 1], F32, name="ss", tag="ss")
                nc.scalar.activation(out=scr0, in_=xt, func=AF.Square,
                                     accum_out=ss)
                # rms = 1/sqrt(ss/2048 + 1e-6)
                sqv = st.tile([128, 1], F32, name="sqv", tag="sqv")
                nc.scalar.activation(out=sqv, in_=ss, func=AF.Sqrt,
                                     bias=epsb, scale=1.0 / DIN)
                rms = st.tile([128, 1], F32, name="rms", tag="rms")
                nc.vector.reciprocal(out=rms, in_=sqv)
                mx = st.tile([128, 1], F32, name="mx", tag="mx")
                nc.vector.reduce_max(out=mx, in_=xgt, axis=AX.X,
                                     apply_absolute_value=True)
                # d = rms*mx + 1e-5 ; s = (127*rms)/d ; s16 = s/16
                d = st.tile([128, 1], F32, name="d", tag="d")
                nc.vector.scalar_tensor_tensor(out=d, in0=rms, scalar=mx,
                                               in1=eps5, op0=AL.mult,
                                               op1=AL.add)
                rcd = st.tile([128, 1], F32, name="rcd", tag="rcd")
                nc.vector.reciprocal(out=rcd, in_=d)
                s = st.tile([128, 1], F32, name="s", tag="s")
                nc.vector.scalar_tensor_tensor(out=s, in0=rcd, scalar=127.0,
                                               in1=rms, op0=AL.mult,
                                               op1=AL.mult)
                s16 = st.tile([128, 1], F32, name="s16", tag="s16")
                nc.vector.tensor_scalar(out=s16, in0=s, scalar1=0.0625,
                                        scalar2=None, op0=AL.mult)

                # scr1 = xg*s + C (= C + q, exact RNE via ACT fma) and, IN
                # PARALLEL on Pool, scr2 = xg*s16 + C (= C + qh)
                scr1 = s1p.tile([128, DIN], F32, name="scr1")
                nc.scalar.activation(out=scr1, in_=xgt, func=AF.Identity,
                                     bias=cmag, scale=s)
                scr2 = s2p.tile([128, DIN], F32, name="scr2")
                nc.gpsimd.tensor_scalar(out=scr2, in0=xgt, scalar1=s16,
                                        scalar2=C_MAGIC, op0=AL.mult,
                                        op1=AL.add)
                # packed fp8 pair planes: lane0 = 16*qh, lane1 = q - 16*qh
                qpk = qp.tile([128, 2 * DIN], FP8, name="qpk")
                nc.vector.tensor_scalar(out=qpk[:, 0::2], in0=scr2,
                                        scalar1=C_MAGIC, scalar2=16.0,
                                        op0=AL.subtract, op1=AL.mult)
                nc.vector.scalar_tensor_tensor(out=qpk[:, 1::2], in0=scr1,
                                               scalar=C_MAGIC,
                                               in1=qpk[:, 0::2],
                                               op0=AL.subtract,
                                               op1=AL.subtract)

                # k-major planes via xbar transpose of the u16 pair view
                qtp = qtpool.tile([128, KB, 128], U16, name="qtp")
                nc.sync.dma_start_transpose(qtp, qpk[:].bitcast(U16))

                if it == 0:
                    for kt in range(2, KB):
                        dma_wq(kt)

                # matmul: out[t, o] = sum_k (16qh + lo)[k, t] * w[k, o]
                # matmuls run og-PAIR-outer into psum halves [128,1024];
                # the evacuation is delayed ONE tile (psum holds exactly two
                # tiles = 8 banks) so evac ops are ready when dispatched and
                # never clog the in-order ACT wait queue behind a PE wait
                qf = qtp[:].bitcast(FP8)  # [128, KB, 256]
                phs = []
                for h in range(2):
                    ph = psp.tile([128, 2 * OGW], F32, name=f"ph{h}",
                                  tag=f"ph{h}")
                    phs.append(ph)
                    for ogl in range(2):
                        og = 2 * h + ogl
                        for kt in range(KB):
                            rhs = (wq[:, kt, og * OGW:(og + 1) * OGW]
                                   .unsqueeze(1).broadcast_to((128, 2, OGW)))
                            nc.tensor.matmul(
                                ph[:, ogl * OGW:(ogl + 1) * OGW],
                                lhsT=qf[:, kt, :], rhs=rhs,
                                start=(kt == 0), stop=(kt == KB - 1),
                                perf_mode=PM.DoubleRowSwInterleave)
                if prev is not None:
                    pphs, pit = prev
                    obuf = op.tile([128, DOUT], BF16, name="obuf")
                    for h in range(2):
                        nc.scalar.copy(
                            out=obuf[:, h * 2 * OGW:(h + 1) * 2 * OGW],
                            in_=pphs[h][:])
                    nc.sync.dma_start(out=o_d[pit * 128:(pit + 1) * 128, :],
                                      in_=obuf)
                prev = (phs, it)

            # drain the last tile
            pphs, pit = prev
            obuf = op.tile([128, DOUT], BF16, name="obuf")
            for h in range(2):
                nc.scalar.copy(out=obuf[:, h * 2 * OGW:(h + 1) * 2 * OGW],
                               in_=pphs[h][:])
            nc.sync.dma_start(out=o_d[pit * 128:(pit + 1) * 128, :],
                              in_=obuf)

    nc.compile()
    return nc


def _rev128(a):
    """Reverse rows within each 128-row block (absorbs the SwInterleave
    stationary-column reversal)."""
    n = a.shape[0]
    return np.ascontiguousarray(
        a.reshape(n // 128, 128, -1)[:, ::-1, :].reshape(a.shape))


def kernel(x, gamma, W):
    import ml_dtypes

    x = np.asarray(x, dtype=np.float32)
    gamma = np.asarray(gamma, dtype=np.float32)
    W = np.asarray(W, dtype=np.float32)

    # ---- host: ternary weight quant (reference fp32 semantics) ----
    m = np.float32(np.abs(W).astype(np.float64).mean())
    denom = np.float32(m + np.float32(1e-5))
    ws = np.float32(np.float32(1.0) / denom)
    inv_ws = np.float32(np.float32(1.0) / ws)
    wqh = np.clip(np.rint((W * ws).astype(np.float32)), -1.0, 1.0)
    # k-major [128, KB*DOUT]: element (p, kt*DOUT+o) = wq[kt*128+p, o]
    wk = np.ascontiguousarray(
        wqh.T.reshape(KB, 128, DOUT).transpose(1, 0, 2).reshape(128, KB * DOUT)
    ).astype(ml_dtypes.float8_e4m3)

    xf = x.reshape(TOK, DIN)
    xb = xf.astype(ml_dtypes.bfloat16)          # device input precision
    use_xg = not np.all(gamma == np.float32(1.0))
    if use_xg:
        xgb = (xf * gamma[None, :]).astype(ml_dtypes.bfloat16)

    key = ("nc", use_xg)
    if key not in _CACHE:
        _CACHE[key] = _build(use_xg)
    nc = _CACHE[key]

    xr = _rev128(xb)
    in_maps = []
    for c in range(NCORES):
        im = {"x": xr[c * TPC:(c + 1) * TPC], "wq": wk}
        if use_xg:
            im["xg"] = _rev128(xgb)[c * TPC:(c + 1) * TPC]
        in_maps.append(im)
    res = run_bass_kernel_spmd(nc, in_maps, list(range(NCORES)))
    out_int = np.concatenate(
        [res.results[c]["out"].astype(np.float32) for c in range(NCORES)],
        axis=0)

    # ---- host: per-token output scale s3 = (1/a)/ws ----
    xbf = xb.astype(np.float32)
    if use_xg:
        xng = xgb.astype(np.float32)
    else:
        xng = xbf
    ss = (xbf.astype(np.float64) ** 2).sum(axis=1)
    sqv = np.sqrt((ss / DIN + 1e-6).astype(np.float32)).astype(np.float32)
    rms = (np.float32(1.0) / sqv).astype(np.float32)
    mx = np.abs(xng).max(axis=1)
    d = ((rms * mx).astype(np.float32) + np.float32(1e-5)).astype(np.float32)
    a = (np.float32(127.0) * (np.float32(1.0) / d).astype(np.float32)
         ).astype(np.float32)
    s3 = ((np.float32(1.0) / a).astype(np.float32) * inv_ws).astype(np.float32)

    out = out_int * s3[:, None]
    return out.reshape(B, S, DOUT).astype(np.float32)


if __name__ == "__main__":
    rng = np.random.default_rng(0)
    x = rng.standard_normal((B, S, DIN), dtype=np.float32)
    gamma = np.ones((DIN,), dtype=np.float32)
    bound = 1.0 / np.sqrt(DIN)
    W = rng.uniform(-bound, bound, (DOUT, DIN)).astype(np.float32)
    out = kernel(x, gamma, W)
    print("out", out.shape, out.dtype, float(np.abs(out).mean()))
